# revision 2
# baseline (speedup 1.0000x reference)
"""Distributed Trainium2 kernel for masked node-MLP update (GNN message passing).

Problem: out = node_tensor, with rows listed in `partition` replaced by
    y = relu(x @ W1 + b1) @ W2 + b2   (x = node_tensor[partition])

Only the P = |partition| gathered rows touch the device: passthrough rows
are copied host-side (out = node_tensor.copy(); out[partition] = y + b2).
The device kernel is a dense MLP over the gathered rows, data-parallel
across 8 cores, activations shipped TRANSPOSED (xT: [D, rows]) in
fp8_e3m4 BOTH directions (measured full-output rel err ~1.1e-2 < 2e-2).

v2 vs v1 (156.5us): rows padded to 126976/core so every matmul is an
exactly-bank-sized 512-col f32 write and every relu/evac is ONE
CONTIGUOUS 1024-col op (v1 used 500-col matmuls at 512-aligned offsets,
forcing strided [p,2,500] pointwise views).  b1 == 0 for this problem, so
the relu drops the bias operand (plain Relu on ACT / tensor_scalar_max
on DVE).  Engine mix rebalanced for the contiguous-op costs (ACT 0.974
vs DVE ~1.17 ns/col).  First DMA block shrunk to 1 unit so the PE/ACT
pipe starts ~2us earlier.

Per-core pipeline (124 units of 1024 cols; DMA blocks 1+3+30x4 units):
    DMA : xT block in, yT block out                 (~108 us busy)
    PE  : z = W1^T x ; y = W2^T h   (4x 512-col MM) (~110-127 us)
    ACT+DVE (mixed): relu z->h bf16, evac y->f8     (~136 us each)
"""

import sys

sys.path.insert(0, "/opt/trn_rl_repo")

import numpy as np
import ml_dtypes

import concourse.bass as bass
import concourse.tile as tile
from concourse import bacc, mybir
from concourse.bass_utils import run_bass_kernel_spmd

D = 128
NCORES = 8
SUB = 512                  # matmul chunk = one full f32 PSUM bank
UNIT = 2 * SUB             # pointwise op granularity (contiguous, 2 banks)
ROWS = 126976              # 124 units; P/8=125000 padded up 1.6%
NUNITS = ROWS // UNIT      # 124
# DMA blocks in units: small first block for a fast pipeline start
BLOCK_UNITS = [1, 3] + [4] * 30
assert sum(BLOCK_UNITS) == NUNITS

BF16 = mybir.dt.bfloat16
F32 = mybir.dt.float32
F8 = mybir.dt.float8e4
F8E3 = mybir.dt.float8e3

_DT = {"bf16": BF16, "f8": F8, "f8e3": F8E3}
_NPDT = {"bf16": ml_dtypes.bfloat16, "f8": ml_dtypes.float8_e4m3,
         "f8e3": ml_dtypes.float8_e3m4}

# x and y both ship as fp8_e3m4 (4 mantissa bits, range +-15.9 — plenty
# for this problem's unit-scale data), halving HBM traffic vs bf16 on both
# sides. Measured rel err ~1.1e-2 < 2e-2 gate. "bf16" is the ~2e-3 fallback.
X_DTYPE = "f8e3"
Y_DTYPE = "f8e3"

# fraction of relu/evac ops assigned to ACT (rest on DVE): balances
# ACT ~1.00us vs DVE ~1.20us per contiguous 1024-col PSUM op.
ACT_SHARE = 0.545

_cache = {}

# test-harness knobs: set TRACE=True before calling kernel() to capture a
# neuron profile; the BassKernelResults lands in LAST_RESULT.
TRACE = False
LAST_RESULT = None


def _build(x_dtype: str, y_dtype: str, b1_zero: bool):
    """Build + compile the SPMD program for a ROWS-row shard per core."""
    XDT = _DT[x_dtype]
    YDT = _DT[y_dtype]

    nc = bacc.Bacc("TRN2", target_bir_lowering=False, debug=False,
                   num_devices=NCORES)

    xT = nc.declare_dram_parameter("xT", [D, ROWS], XDT, isOutput=False)
    w1 = nc.declare_dram_parameter("w1", [D, D], BF16, isOutput=False)
    w2 = nc.declare_dram_parameter("w2", [D, D], BF16, isOutput=False)
    b1c = nc.declare_dram_parameter("b1c", [D, 1], F32, isOutput=False)
    out = nc.declare_dram_parameter("out", [D, ROWS], YDT, isOutput=True)

    # unit -> (block index, column offset of unit within block)
    unit_block = []
    block_cols = []           # (start_col, ncols) per block
    col = 0
    for b, nu in enumerate(BLOCK_UNITS):
        block_cols.append((col, nu * UNIT))
        for u in range(nu):
            unit_block.append((b, u * UNIT))
        col += nu * UNIT

    with tile.TileContext(nc) as tc:
        with (
            tc.tile_pool(name="consts", bufs=1) as consts,
            tc.tile_pool(name="io", bufs=8) as io,
            tc.tile_pool(name="small", bufs=4) as small,
            tc.tile_pool(name="psum_h", bufs=2, space="PSUM") as psum_h_pool,
            tc.tile_pool(name="psum_o", bufs=2, space="PSUM") as psum_o_pool,
        ):
            # first x block issued before the consts so the input stream
            # starts flowing at t=0 of the DMA pipe.
            xt_tiles = {}     # block -> xT sbuf tile
            out_tiles = {}    # block -> out sbuf tile
            h_t = {}          # unit -> hidden tile [D, UNIT]

            def load_block(b):
                st, ncols = block_cols[b]
                xt_t = io.tile([D, ncols], XDT, tag="xin", name=f"xt_{b}")
                nc.sync.dma_start(out=xt_t, in_=xT[:, st:st + ncols])
                xt_tiles[b] = xt_t
                out_tiles[b] = io.tile([D, ncols], YDT, tag="xout",
                                       name=f"ot_{b}")

            load_block(0)

            w1_s = consts.tile([D, D], BF16)
            nc.sync.dma_start(out=w1_s, in_=w1[:, :])
            w2_s = consts.tile([D, D], BF16)
            nc.sync.dma_start(out=w2_s, in_=w2[:, :])
            b1_s = consts.tile([D, 1], F32)
            nc.sync.dma_start(out=b1_s, in_=b1c[:, :])

            # Mixed ACT/DVE assignment for the relu/evac ops: spreading each
            # unit's chain across both engines decorrelates the PE queue's
            # cross-engine waits (strict per-op-type assignment measured
            # slower in v1).
            _eng_acc = [0.0]

            def pick_engine():
                _eng_acc[0] += ACT_SHARE
                if _eng_acc[0] >= 1.0:
                    _eng_acc[0] -= 1.0
                    return "act"
                return "dve"

            def stage_a(j):  # PE: 2x mm1 ; ACT or DVE: relu over unit
                b, off = unit_block[j]
                ph = psum_h_pool.tile([D, UNIT], F32, tag="ph", name=f"ph_{j}")
                xt = xt_tiles[b]
                for half in range(2):
                    nc.tensor.matmul(
                        out=ph[:, half * SUB:(half + 1) * SUB],
                        lhsT=w1_s,
                        rhs=xt[:, off + half * SUB:off + (half + 1) * SUB],
                        start=True, stop=True)
                h = small.tile([D, UNIT], BF16, tag="h", name=f"h_{j}")
                if pick_engine() == "act":
                    if b1_zero:
                        nc.scalar.activation(
                            h[:, :], ph[:, :],
                            mybir.ActivationFunctionType.Relu)
                    else:
                        nc.scalar.activation(
                            h[:, :], ph[:, :],
                            mybir.ActivationFunctionType.Relu,
                            bias=b1_s[:, :])
                else:
                    if b1_zero:
                        nc.vector.tensor_scalar_max(h[:, :], ph[:, :], 0.0)
                    else:
                        nc.vector.tensor_scalar(out=h[:, :], in0=ph[:, :],
                                                scalar1=b1_s[:, :],
                                                scalar2=0.0,
                                                op0=mybir.AluOpType.add,
                                                op1=mybir.AluOpType.max)
                h_t[j] = h

            def stage_b(j):  # PE: 2x mm2 ; DVE or ACT: evac (cast to f8)
                b, off = unit_block[j]
                po = psum_o_pool.tile([D, UNIT], F32, tag="po", name=f"po_{j}")
                h = h_t.pop(j)
                for half in range(2):
                    nc.tensor.matmul(
                        out=po[:, half * SUB:(half + 1) * SUB],
                        lhsT=w2_s,
                        rhs=h[:, half * SUB:(half + 1) * SUB],
                        start=True, stop=True)
                # b2 is folded into the host-side scatter: evac is a pure
                # copy+downcast.
                ot = out_tiles[b]
                if pick_engine() == "act":
                    nc.scalar.activation(ot[:, off:off + UNIT], po[:, :],
                                         mybir.ActivationFunctionType.Copy)
                else:
                    nc.vector.tensor_copy(ot[:, off:off + UNIT], po[:, :])
                st, ncols = block_cols[b]
                last_unit_of_block = (off + UNIT == ncols)
                if b == len(BLOCK_UNITS) - 1:
                    # fine-grained stores at the very end shorten the drain
                    nc.sync.dma_start(out=out[:, st + off:st + off + UNIT],
                                      in_=ot[:, off:off + UNIT])
                elif last_unit_of_block:
                    nc.sync.dma_start(out=out[:, st:st + ncols], in_=ot)
                if last_unit_of_block:
                    del xt_tiles[b], out_tiles[b]

            # prefetch runs PF_UNITS of compute ahead; SKEW delays stage_b
            SKEW = 2
            PF_UNITS = 16
            next_block = 1
            # cumulative first-unit index per block
            block_first_unit = []
            u = 0
            for nu in BLOCK_UNITS:
                block_first_unit.append(u)
                u += nu

            for j in range(NUNITS + SKEW):
                while (next_block < len(BLOCK_UNITS)
                       and block_first_unit[next_block] <= j + PF_UNITS):
                    load_block(next_block)
                    next_block += 1
                if j < NUNITS:
                    stage_a(j)
                if 0 <= j - SKEW < NUNITS:
                    stage_b(j - SKEW)

    nc.compile()
    return nc


def _get_nc(x_dtype: str, y_dtype: str, b1_zero: bool):
    key = (x_dtype, y_dtype, b1_zero)
    if key not in _cache:
        _cache[key] = _build(x_dtype, y_dtype, b1_zero)
    return _cache[key]


def kernel(node_tensor, W1, b1, W2, b2, partition):
    node_tensor = np.asarray(node_tensor, dtype=np.float32)
    W1 = np.asarray(W1, dtype=np.float32)
    b1 = np.asarray(b1, dtype=np.float32)
    W2 = np.asarray(W2, dtype=np.float32)
    b2 = np.asarray(b2, dtype=np.float32)
    partition = np.asarray(partition)

    n, d = node_tensor.shape
    p = partition.shape[0]
    assert d == D and p <= NCORES * ROWS, (n, d, p)

    bf = ml_dtypes.bfloat16
    consts = {
        "w1": W1.astype(bf),
        "w2": W2.astype(bf),
        "b1c": b1.reshape(D, 1).astype(np.float32),
    }

    # gather the partition rows host-side; only they touch the device
    xg = node_tensor[partition].astype(_NPDT[X_DTYPE])   # [P, D]
    pad = NCORES * ROWS - p
    if pad:
        xg = np.concatenate(
            [xg, np.zeros((pad, D), dtype=_NPDT[X_DTYPE])], axis=0)
    in_maps = []
    for i in range(NCORES):
        sl = slice(i * ROWS, (i + 1) * ROWS)
        in_maps.append({
            "xT": np.ascontiguousarray(xg[sl].T),   # [D, ROWS]
            **consts,
        })

    nc = _get_nc(X_DTYPE, Y_DTYPE, not np.any(b1 != 0.0))
    res = run_bass_kernel_spmd(nc, in_maps, list(range(NCORES)), trace=TRACE)
    global LAST_RESULT
    LAST_RESULT = res

    y = np.empty((NCORES * ROWS, D), dtype=_NPDT[Y_DTYPE])
    for i in range(NCORES):
        y[i * ROWS:(i + 1) * ROWS] = res.results[i]["out"].T

    yf = y[:p].astype(np.float32)
    yf += b2[None, :]          # b2 folded here instead of on-device
    out = node_tensor.copy()
    out[partition] = yf
    return out


if __name__ == "__main__":
    # small self-test with the full-size program (padded rows)
    rng = np.random.default_rng(0)
    n_small = 1_200_000
    p_small = 1_000_000
    nt = rng.standard_normal((n_small, D), dtype=np.float32)
    W1t = (rng.standard_normal((D, D), dtype=np.float32) / np.sqrt(D))
    b1t = np.zeros(D, dtype=np.float32)
    W2t = (rng.standard_normal((D, D), dtype=np.float32) / np.sqrt(D))
    b2t = rng.standard_normal(D).astype(np.float32) * 0.01
    part = rng.permutation(n_small)[:p_small].astype(np.int32)

    outv = kernel(nt, W1t, b1t, W2t, b2t, part)

    x = nt[part]
    y = np.maximum(x @ W1t + b1t, 0.0) @ W2t + b2t
    ref = nt.copy()
    ref[part] = y
    err = np.linalg.norm(outv - ref) / np.linalg.norm(ref)
    keep = ~np.isin(np.arange(n_small), part)
    exact = np.array_equal(outv[keep], ref[keep])
    print("rel_err:", err, "passthrough exact:", exact)
